# revision 6
# baseline (speedup 1.0000x reference)
"""DLinear (causal sliding-window-mean decomposition + two linear heads) on 8 TRN2 NeuronCores.

Math: out = trend @ tW.T + seasonal @ sW.T + (tb + sb), seasonal = x - trend,
trend[:, j] = mean(x[:, max(0, j-24):j+1]) (window 25, causal).

trend is linear in x: trend = x @ A with A[i, j] = 1/c(j) for j-24 <= i <= j,
c(j) = min(j+1, 25). Folding:
    out = x @ W_eff + (tb + sb),  W_eff = sW.T + A @ (tW - sW).T
so the sliding-window work lands on the small [720, 2048] weight delta instead of
x, and the x-side is a single [B, S] @ [S, O] matmul.

Sharding: batch 8-way (512 rows/core); weights replicated; W_eff computed
(redundantly) on every core via banded matmuls on the TensorE. No collectives.

Device layout: the contraction (S) must sit on SBUF partitions for the TensorE,
so the host passes x.T (per-core column slice) and the two transposed weight
matrices packed block-interleaved — layout/dtype prep only, all arithmetic is
on device. Inputs are fed as fp16: its 11-bit mantissa matches the TensorE's
own single-pass fp32 (f32r) rounding, so accuracy stays at the few-1e-4 level
(measured) while DMA bytes halve; accumulation is fp32 in PSUM throughout.
"""

import sys

sys.path.insert(0, "/opt/trn_rl_repo")

import numpy as np

import concourse.bacc as bacc
import concourse.mybir as mybir
from concourse.tile import TileContext
from concourse.bass_utils import run_bass_kernel_spmd

B, S, O = 4096, 2048, 720
WIN = 25
NCORES = 8
BC = B // NCORES          # batch rows per core
NK = S // 128             # 16 S-blocks of 128

F32 = mybir.dt.float32
F16 = mybir.dt.float16

_nc_cache = None


def _build_bands():
    """Band matrices as matmul lhsT ([K=j, M=i]): G^T[i,o] = sum_j band[j,i] * D^T[j,o].

    b0f: within-block band for S-block 0, carries 1/c(j) = 1/min(j+1, 25).
    b0r: within-block band for blocks >= 1, carries 1/25.
    b1:  next-block band (rows j2 of block k+1 contribute to i >= 104+j2), 1/25.
    """
    b0f = np.zeros((128, 128), np.float32)
    b0r = np.zeros((128, 128), np.float32)
    b1 = np.zeros((128, 128), np.float32)
    for i in range(128):
        for j in range(i, min(i + WIN, 128)):
            b0f[j, i] = 1.0 / min(j + 1, WIN)
            b0r[j, i] = 1.0 / WIN
        for j2 in range(0, i - 104 + 1):
            b1[j2, i] = 1.0 / WIN
    return b0f.astype(np.float16), b0r.astype(np.float16), b1.astype(np.float16)


def build_in_maps(x, trend_W, trend_b, seasonal_W, seasonal_b):
    """Host-side shard + layout/dtype prep. Returns per-core input dicts."""
    x16 = np.asarray(x, dtype=np.float16)
    tR = np.asarray(trend_W, dtype=np.float16).T.reshape(NK, 128, O)
    sR = np.asarray(seasonal_W, dtype=np.float16).T.reshape(NK, 128, O)
    # [128, NK*2*O]: block k occupies cols [1440k, 1440k+1440) = tWT_k || sWT_k
    wpk = np.ascontiguousarray(
        np.concatenate([tR, sR], axis=2).transpose(1, 0, 2).reshape(128, NK * 2 * O)
    )
    bsum = np.ascontiguousarray(
        (np.asarray(trend_b, np.float32) + np.asarray(seasonal_b, np.float32)).reshape(O, 1)
    )
    b0f, b0r, b1 = _build_bands()

    xT = x16.T  # [S, B] view
    in_maps = []
    for i in range(NCORES):
        xc = np.ascontiguousarray(xT[:, i * BC : (i + 1) * BC])  # [S, BC]
        # [128, NK*BC]: block k occupies cols [BC*k, BC*(k+1))
        xpk = np.ascontiguousarray(
            xc.reshape(NK, 128, BC).transpose(1, 0, 2).reshape(128, NK * BC)
        )
        in_maps.append(
            {"xpk": xpk, "wpk": wpk, "bsum": bsum, "b0f": b0f, "b0r": b0r, "b1": b1}
        )
    return in_maps


def _build_nc():
    nc = bacc.Bacc()
    xpk = nc.declare_dram_parameter("xpk", [128, NK * BC], F16, isOutput=False)
    wpk = nc.declare_dram_parameter("wpk", [128, NK * 2 * O], F16, isOutput=False)
    bsum = nc.declare_dram_parameter("bsum", [O, 1], F32, isOutput=False)
    b0f = nc.declare_dram_parameter("b0f", [128, 128], F16, isOutput=False)
    b0r = nc.declare_dram_parameter("b0r", [128, 128], F16, isOutput=False)
    b1 = nc.declare_dram_parameter("b1", [128, 128], F16, isOutput=False)
    outT = nc.declare_dram_parameter("outT", [O, BC], F32, isOutput=True)

    NOT = (O + 127) // 128  # 6 output-column tiles (5x128 + 80)
    XCH = 4                 # x loaded in 4 chunks of 4 blocks

    with TileContext(nc) as tc:
        with (
            tc.tile_pool(name="consts", bufs=1) as consts,
            tc.tile_pool(name="wlp", bufs=3) as wlp,
            tc.tile_pool(name="dp", bufs=3) as dp,
            tc.tile_pool(name="wp", bufs=3) as wp,
            tc.tile_pool(name="op", bufs=3) as op,
            tc.tile_pool(name="pw", bufs=1, space="PSUM") as pwp,
            tc.tile_pool(name="po", bufs=1, space="PSUM") as pop,
        ):
            # constants
            b0f_t = consts.tile([128, 128], F16, tag="b0f")
            b0r_t = consts.tile([128, 128], F16, tag="b0r")
            b1_t = consts.tile([128, 128], F16, tag="b1")
            nc.sync.dma_start(out=b0f_t[:], in_=b0f[:])
            nc.sync.dma_start(out=b0r_t[:], in_=b0r[:])
            nc.sync.dma_start(out=b1_t[:], in_=b1[:])
            bs_t = []
            for ot in range(NOT):
                o0, ow = 128 * ot, min(128, O - 128 * ot)
                t = consts.tile([128, 1], F32, tag=f"bs{ot}", name=f"bs{ot}")
                nc.sync.dma_start(out=t[0:ow, :], in_=bsum[o0 : o0 + ow, :])
                bs_t.append(t)

            # all of x stays resident: [128, NK*BC] fp16 = 2 MB, loaded in 4 chunks
            xall = consts.tile([128, NK * BC], F16, tag="xall")
            ch = NK * BC // XCH
            for c in range(XCH):
                nc.sync.dma_start(
                    out=xall[:, c * ch : (c + 1) * ch], in_=xpk[:, c * ch : (c + 1) * ch]
                )

            # persistent psum accumulators: 6 banks out + 2 banks W-prep = 8
            po_t = [pop.tile([128, BC], F32, tag=f"po{ot}", name=f"po{ot}") for ot in range(NOT)]
            pw_t = pwp.tile([128, O], F32, tag="pw")

            sw_t, d_t = {}, {}
            for k in range(NK + 1):
                if k < NK:
                    # weight block load (tWT_k || sWT_k) and D_k = tWT_k - sWT_k
                    wl = wlp.tile([128, 2 * O], F16, tag="wl", name=f"wl{k}")
                    nc.sync.dma_start(
                        out=wl[:], in_=wpk[:, 2 * O * k : 2 * O * (k + 1)]
                    )
                    sw_t[k] = wl[:, O : 2 * O]
                    d_t[k] = dp.tile([128, O], F16, tag="d", name=f"d{k}")
                    nc.vector.tensor_tensor(
                        out=d_t[k][:],
                        in0=wl[:, 0:O],
                        in1=sw_t[k],
                        op=mybir.AluOpType.subtract,
                    )
                if k >= 1:
                    j = k - 1
                    # banded matmuls: pw = b0 @ D_j (+ b1 @ D_{j+1})
                    b0 = b0f_t if j == 0 else b0r_t
                    last = j == NK - 1
                    for n0, n1 in ((0, 512), (512, O)):
                        nc.tensor.matmul(
                            pw_t[:, n0:n1], b0[:], d_t[j][:, n0:n1],
                            start=True, stop=last,
                        )
                    if not last:
                        for n0, n1 in ((0, 512), (512, O)):
                            nc.tensor.matmul(
                                pw_t[:, n0:n1], b1_t[:], d_t[j + 1][:, n0:n1],
                                start=False, stop=True,
                            )
                    # W_eff^T_j = pw + sWT_j   (PSUM evac on DVE)
                    w = wp.tile([128, O], F16, tag="w", name=f"w{j}")
                    nc.vector.tensor_tensor(
                        out=w[:], in0=pw_t[:], in1=sw_t[j],
                        op=mybir.AluOpType.add,
                    )
                    # main: out^T[o_tile] += W_eff^T_j[:, o_slice].T @ xT_j
                    for ot in range(NOT):
                        o0, ow = 128 * ot, min(128, O - 128 * ot)
                        nc.tensor.matmul(
                            po_t[ot][0:ow, :],
                            w[:, o0 : o0 + ow],
                            xall[:, BC * j : BC * (j + 1)],
                            start=(j == 0),
                            stop=(j == NK - 1),
                        )
            # epilogue: bias add fused into PSUM evac on ScalarE, then store
            for ot in range(NOT):
                o0, ow = 128 * ot, min(128, O - 128 * ot)
                osb = op.tile([128, BC], F32, tag="o", name=f"osb{ot}")
                nc.scalar.activation(
                    out=osb[0:ow, :], in_=po_t[ot][0:ow, :],
                    func=mybir.ActivationFunctionType.Identity, bias=bs_t[ot][0:ow, :],
                )
                nc.sync.dma_start(out=outT[o0 : o0 + ow, :], in_=osb[0:ow, :])

    nc.compile()
    return nc


def kernel(x, trend_W, trend_b, seasonal_W, seasonal_b):
    global _nc_cache
    if _nc_cache is None:
        _nc_cache = _build_nc()
    in_maps = build_in_maps(x, trend_W, trend_b, seasonal_W, seasonal_b)
    res = run_bass_kernel_spmd(_nc_cache, in_maps, list(range(NCORES)))
    out = np.concatenate([r["outT"] for r in res.results], axis=1)  # [O, B]
    return np.ascontiguousarray(out.T)


# revision 7
# speedup vs baseline: 1.3965x; 1.3965x over previous
"""DLinear (causal sliding-window-mean decomposition + two linear heads) on 8 TRN2 NeuronCores.

Math: out = trend @ tW.T + seasonal @ sW.T + (tb + sb), seasonal = x - trend,
trend[:, j] = mean(x[:, max(0, j-24):j+1]) (window 25, causal).

trend is linear in x: trend = x @ A with A[i, j] = 1/c(j) for j-24 <= i <= j,
c(j) = min(j+1, 25). Folding:
    out = x @ W_eff + (tb + sb),  W_eff = sW.T + A @ (tW - sW).T
so the sliding-window work lands on the small [720, 2048] weight delta instead
of x, and the x-side is a single [B, S] @ [S, O] matmul.

Sharding: 2D, 4-way batch x 2-way output: core i handles batch rows
[1024*(i%4), ...) and output columns [360*(i//4), ...). Each core builds its
W_eff half via banded matmuls on the TensorE (b0/b1 carry the 1/count
normalization); halving O per core halves the W-prep and lets the W-prep PSUM
tile fit one bank, so it double-buffers alongside the 6 out-accumulator banks
(8 total). No collectives.

Device layout: the contraction (S) must sit on SBUF partitions for the
TensorE, so the host passes x.T slices and transposed weight halves packed
block-interleaved - layout/dtype prep only, all arithmetic is on device.
Inputs are fed as fp16: its 11-bit mantissa matches the TensorE's own
single-pass fp32 (f32r) rounding, so accuracy stays at the few-1e-4 level
(measured) while DMA bytes halve; accumulation is fp32 in PSUM throughout.
x streams on the ScalarE HWDGE queues, weights on the SyncE queues, so
neither load delays the other.
"""

import sys

sys.path.insert(0, "/opt/trn_rl_repo")

import numpy as np

import concourse.bacc as bacc
import concourse.mybir as mybir
from concourse.tile import TileContext
from concourse.bass_utils import run_bass_kernel_spmd

B, S, O = 4096, 2048, 720
WIN = 25
NCORES = 8
NBG, NOG = 4, 2           # batch groups x output groups
BC = B // NBG             # 1024 batch rows per core
OC = O // NOG             # 360 output cols per core
NK = S // 128             # 16 S-blocks of 128
WBLK = 2 * OC             # 720 packed weight cols per S-block (tW-half || sW-half)

F32 = mybir.dt.float32
F16 = mybir.dt.float16

_nc_cache = None


def _build_bands():
    """Band matrices as matmul lhsT ([K=j, M=i]): G^T[i,o] = sum_j band[j,i] * D^T[j,o].

    b0f: within-block band for S-block 0, carries 1/c(j) = 1/min(j+1, 25).
    b0r: within-block band for blocks >= 1, carries 1/25.
    b1:  next-block band (rows j2 of block k+1 contribute to i >= 104+j2), 1/25.
    """
    b0f = np.zeros((128, 128), np.float32)
    b0r = np.zeros((128, 128), np.float32)
    b1 = np.zeros((128, 128), np.float32)
    for i in range(128):
        for j in range(i, min(i + WIN, 128)):
            b0f[j, i] = 1.0 / min(j + 1, WIN)
            b0r[j, i] = 1.0 / WIN
        for j2 in range(0, i - 104 + 1):
            b1[j2, i] = 1.0 / WIN
    return b0f.astype(np.float16), b0r.astype(np.float16), b1.astype(np.float16)


def build_in_maps(x, trend_W, trend_b, seasonal_W, seasonal_b):
    """Host-side shard + layout/dtype prep. Returns per-core input dicts."""
    x16 = np.asarray(x, dtype=np.float16)
    tT = np.asarray(trend_W, dtype=np.float16).T      # [S, O]
    sT = np.asarray(seasonal_W, dtype=np.float16).T
    bs = (np.asarray(trend_b, np.float32) + np.asarray(seasonal_b, np.float32)).reshape(O, 1)
    b0f, b0r, b1 = _build_bands()

    # weight packs per output half: [128, NK*720], block k = tT half || sT half
    wpks = []
    for g in range(NOG):
        tR = tT[:, g * OC : (g + 1) * OC].reshape(NK, 128, OC)
        sR = sT[:, g * OC : (g + 1) * OC].reshape(NK, 128, OC)
        wpks.append(
            np.ascontiguousarray(
                np.concatenate([tR, sR], axis=2).transpose(1, 0, 2).reshape(128, NK * WBLK)
            )
        )
    bsums = [np.ascontiguousarray(bs[g * OC : (g + 1) * OC]) for g in range(NOG)]

    xT = x16.T  # [S, B] view
    xpks = []
    for c in range(NBG):
        xc = np.ascontiguousarray(xT[:, c * BC : (c + 1) * BC])  # [S, BC]
        xpks.append(
            np.ascontiguousarray(
                xc.reshape(NK, 128, BC).transpose(1, 0, 2).reshape(128, NK * BC)
            )
        )

    in_maps = []
    for i in range(NCORES):
        g, c = i // NBG, i % NBG
        in_maps.append(
            {"xpk": xpks[c], "wpk": wpks[g], "bsum": bsums[g],
             "b0f": b0f, "b0r": b0r, "b1": b1}
        )
    return in_maps


def _build_nc():
    nc = bacc.Bacc()
    xpk = nc.declare_dram_parameter("xpk", [128, NK * BC], F16, isOutput=False)
    wpk = nc.declare_dram_parameter("wpk", [128, NK * WBLK], F16, isOutput=False)
    bsum = nc.declare_dram_parameter("bsum", [OC, 1], F32, isOutput=False)
    b0f = nc.declare_dram_parameter("b0f", [128, 128], F16, isOutput=False)
    b0r = nc.declare_dram_parameter("b0r", [128, 128], F16, isOutput=False)
    b1 = nc.declare_dram_parameter("b1", [128, 128], F16, isOutput=False)
    outT = nc.declare_dram_parameter("outT", [OC, BC], F32, isOutput=True)

    OTS = [(0, 128), (128, 128), (256, OC - 256)]   # o-tiles within the 360 half
    NBH = BC // 512                                  # 2 batch halves (psum N=512)

    with TileContext(nc) as tc:
        with (
            tc.tile_pool(name="consts", bufs=1) as consts,
            tc.tile_pool(name="wlp", bufs=3) as wlp,
            tc.tile_pool(name="xcp", bufs=8) as xcp,
            tc.tile_pool(name="dp", bufs=3) as dp,
            tc.tile_pool(name="wp", bufs=3) as wp,
            tc.tile_pool(name="op", bufs=3) as op,
            tc.tile_pool(name="pw", bufs=2, space="PSUM") as pwp,
            tc.tile_pool(name="po", bufs=1, space="PSUM") as pop,
        ):
            # constants (SyncE queue)
            b0f_t = consts.tile([128, 128], F16, tag="b0f")
            b0r_t = consts.tile([128, 128], F16, tag="b0r")
            b1_t = consts.tile([128, 128], F16, tag="b1")
            nc.sync.dma_start(out=b0f_t[:], in_=b0f[:])
            nc.sync.dma_start(out=b0r_t[:], in_=b0r[:])
            nc.sync.dma_start(out=b1_t[:], in_=b1[:])
            bs_t = []
            for ot, (o0, ow) in enumerate(OTS):
                t = consts.tile([128, 1], F32, tag=f"bs{ot}", name=f"bs{ot}")
                nc.sync.dma_start(out=t[0:ow, :], in_=bsum[o0 : o0 + ow, :])
                bs_t.append(t)

            # weight blocks: 8 chunks x 2 S-blocks on SyncE
            wl_t = {}
            for c in range(NK // 2):
                wl_t[c] = wlp.tile([128, 2 * WBLK], F16, tag="wl", name=f"wl{c}")
                nc.sync.dma_start(
                    out=wl_t[c][:], in_=wpk[:, 2 * WBLK * c : 2 * WBLK * (c + 1)]
                )
            # x: 8 chunks x 2 S-blocks on ScalarE queues (parallel to weights)
            xc_t = {}
            for c in range(NK // 2):
                xc_t[c] = xcp.tile([128, 2 * BC], F16, tag="xc", name=f"xc{c}")
                nc.scalar.dma_start(
                    out=xc_t[c][:], in_=xpk[:, 2 * BC * c : 2 * BC * (c + 1)]
                )

            # psum: 6 out accumulators (1 bank each) + double-buffered W-prep bank
            po_t = {}
            for ot in range(len(OTS)):
                for h in range(NBH):
                    po_t[ot, h] = pop.tile(
                        [128, 512], F32, tag=f"po{ot}_{h}", name=f"po{ot}_{h}"
                    )

            def wslice(k, which):  # tW half (0) or sW half (1) of S-block k
                base = (k % 2) * WBLK + which * OC
                return wl_t[k // 2][:, base : base + OC]

            def xslice(k, h):
                return xc_t[k // 2][:, (k % 2) * BC + 512 * h : (k % 2) * BC + 512 * (h + 1)]

            d_t = {}
            for k in range(NK + 1):
                if k < NK:
                    d_t[k] = dp.tile([128, OC], F16, tag="d", name=f"d{k}")
                    nc.vector.tensor_tensor(
                        out=d_t[k][:], in0=wslice(k, 0), in1=wslice(k, 1),
                        op=mybir.AluOpType.subtract,
                    )
                if k >= 1:
                    j = k - 1
                    # banded matmuls: pw = b0 @ D_j (+ b1 @ D_{j+1})
                    pw = pwp.tile([128, OC], F32, tag="pw", name=f"pw{j}")
                    b0 = b0f_t if j == 0 else b0r_t
                    last = j == NK - 1
                    nc.tensor.matmul(pw[:], b0[:], d_t[j][:], start=True, stop=last)
                    if not last:
                        nc.tensor.matmul(pw[:], b1_t[:], d_t[j + 1][:], start=False, stop=True)
                    # W_eff^T_j = pw + sWT_j   (PSUM evac on DVE)
                    w = wp.tile([128, OC], F16, tag="w", name=f"w{j}")
                    nc.vector.tensor_tensor(
                        out=w[:], in0=pw[:], in1=wslice(j, 1), op=mybir.AluOpType.add
                    )
                    # main: po[ot, h] += W_eff^T_j[:, o_slice].T @ x_j[:, h]
                    for ot, (o0, ow) in enumerate(OTS):
                        for h in range(NBH):
                            nc.tensor.matmul(
                                po_t[ot, h][0:ow, :],
                                w[:, o0 : o0 + ow],
                                xslice(j, h),
                                start=(j == 0),
                                stop=(j == NK - 1),
                            )
            # epilogue: bias add fused into PSUM evac on ScalarE, then store
            for ot, (o0, ow) in enumerate(OTS):
                for h in range(NBH):
                    osb = op.tile([128, 512], F32, tag="o", name=f"osb{ot}_{h}")
                    nc.scalar.activation(
                        out=osb[0:ow, :], in_=po_t[ot, h][0:ow, :],
                        func=mybir.ActivationFunctionType.Identity,
                        bias=bs_t[ot][0:ow, :],
                    )
                    nc.sync.dma_start(
                        out=outT[o0 : o0 + ow, 512 * h : 512 * (h + 1)],
                        in_=osb[0:ow, :],
                    )

    nc.compile()
    return nc


def kernel(x, trend_W, trend_b, seasonal_W, seasonal_b):
    global _nc_cache
    if _nc_cache is None:
        _nc_cache = _build_nc()
    in_maps = build_in_maps(x, trend_W, trend_b, seasonal_W, seasonal_b)
    res = run_bass_kernel_spmd(_nc_cache, in_maps, list(range(NCORES)))
    full = np.empty((O, B), np.float32)
    for i, r in enumerate(res.results):
        g, c = i // NBG, i % NBG
        full[g * OC : (g + 1) * OC, c * BC : (c + 1) * BC] = r["outT"]
    return np.ascontiguousarray(full.T)
